# revision 1
# baseline (speedup 1.0000x reference)
"""Aitchison multi-head attention on 8 trn2 NeuronCores.

Strategy:
- CLR centering is linear -> folded into Wq/Wk + biases on the host (fp64).
- Shard: core c handles batch b=c//4 and 4 heads (feature slice of 256).
  QKV/out projection weights sliced per core; host sums the 4 partial
  output projections per batch and adds bo.
- Device kernel (per core, bf16 matmul operands / fp32 PSUM accum):
    qcT,kcT [256,2048] = W_eff @ x.T (+bias, f on partitions)
    v_pl    [2048, 4x65] = x @ Wv.T + bv with a ones column appended per
            head: the PV matmul (lhsT=[tk,65]) then produces the softmax
            denominator as PSUM row 64 for free -- no separate ones-lhs
            sums matmuls (those were 25% of all PE streaming cycles).
    Per unit (head-pair p, 512-wide q block): 8 score groups, each
    [128,1024] PSUM (1 tk tile x 2 heads) x2 double-buffered; one
    2048-wide exp per group into a per-group [128,2048] bf16 tile
    (bufs=4 rolling window keeps SBUF small).
    PV runs IN-unit, one group behind the exp (chunk g-1 after scores of
    group g), into two per-head [65,512] PSUM banks; banks are copied to
    SBUF right after chunk 7 so the next unit's start=True chain reuses
    them after a ~1-group handoff.
    1/Z via reciprocal_approx_fast (5x plain reciprocal), broadcast on
    GPSIMD, normalize mult on DVE into attnT (bf16).
    out partial = attnT.T @ WoT per 128x512 tile -> DVE copy -> DMA out.
- Emission is software-pipelined: the Q/K/V projections beyond the
  lead-in (kcT[0] + first qcT block) are interleaved into early units as
  PE filler; Wo groups drain during the last units and the tail.
"""
import sys
import types

sys.path.insert(0, "/opt/trn_rl_repo")

import numpy as np
import ml_dtypes

import concourse.bass as bass
import concourse.tile as tile
from concourse import bacc, mybir
from concourse.bass_utils import run_bass_kernel_spmd

B, T, E, H, Dh = 2, 2048, 1024, 16, 64
NCORES = 8
HPC = 4            # heads per core
F = HPC * Dh       # 256 features per core
SCALE = 8.0        # sqrt(Dh)
KC = E // 128      # 8 k-chunks in projections
BF = mybir.dt.bfloat16
F32 = mybir.dt.float32
BF_NP = ml_dtypes.bfloat16


def _install_ntff_hook():
    """trace=True under axon needs antenv.axon_hooks, missing in this image."""
    if "antenv.axon_hooks" in sys.modules:
        return
    try:
        from trn_agent_boot.trn_boot import _ntff_profile_via_ctypes

        hook = _ntff_profile_via_ctypes("/opt/axon/libaxon_pjrt.so")
    except Exception:
        hook = None
    mod = types.ModuleType("antenv.axon_hooks")
    mod.get_axon_ntff_profile_hook = lambda: hook
    sys.modules["antenv.axon_hooks"] = mod


def _emit(tc, io):
    nc = tc.nc
    from contextlib import ExitStack

    ctx = ExitStack()
    with ctx:
        const = ctx.enter_context(tc.tile_pool(name="const", bufs=1))
        xpool = ctx.enter_context(tc.tile_pool(name="x", bufs=24))
        qk = ctx.enter_context(tc.tile_pool(name="qk", bufs=1))
        epool = ctx.enter_context(tc.tile_pool(name="exp", bufs=8))
        spool = ctx.enter_context(tc.tile_pool(name="small", bufs=2))
        opool = ctx.enter_context(tc.tile_pool(name="out", bufs=2))
        ps_a = ctx.enter_context(tc.tile_pool(name="psa", bufs=2, space="PSUM"))
        ps_pv = ctx.enter_context(tc.tile_pool(name="pspv", bufs=1, space="PSUM"))
        ps_b = ctx.enter_context(tc.tile_pool(name="psb", bufs=2, space="PSUM"))

        def load_w(name):
            ts = []
            for kk in range(KC):
                t = const.tile([128, F], BF, name=f"{name}{kk}", tag=f"{name}{kk}")
                nc.sync.dma_start(t[:], io[name][kk * 128:(kk + 1) * 128, :])
                ts.append(t)
            return ts

        def load_b(name):
            ts = []
            for ft in range(2):
                t = const.tile([128, 1], F32, name=f"{name}{ft}", tag=f"{name}{ft}")
                nc.sync.dma_start(t[:], io[name][ft * 128:(ft + 1) * 128, :])
                ts.append(t)
            return ts

        def load_x(which):
            xc = []
            for kk in range(KC):
                t = xpool.tile([128, T], BF, name="xc", tag="xc")
                nc.sync.dma_start(t[:], io[which][kk * 128:(kk + 1) * 128, :])
                xc.append(t)
            return xc

        # ---- persistent activation tiles ----
        qcT = [qk.tile([128, T], BF, name=f"qcT{ft}", tag=f"qcT{ft}") for ft in range(2)]
        kcT = [qk.tile([128, T], BF, name=f"kcT{ft}", tag=f"kcT{ft}") for ft in range(2)]
        attnT = [qk.tile([128, T], BF, name=f"attnT{ft}", tag=f"attnT{ft}") for ft in range(2)]
        # v with a ones column per head: [t, 4 heads x (64 v | 1 one)]
        v_pl = [qk.tile([128, HPC, Dh + 1], BF, name=f"vpl{tt}", tag=f"vpl{tt}")
                for tt in range(16)]

        def proj_qk(wt, bt, dst, xc, ft, tbps=(0, 1, 2, 3)):
            for tbp in tbps:  # 512-wide t groups (psb rotation, no stalls)
                ps = ps_b.tile([128, 512], F32, name="psp", tag="psb")
                tq0 = tbp * 512
                for kk in range(KC):
                    nc.tensor.matmul(
                        ps[:],
                        wt[kk][:, ft * 128:(ft + 1) * 128],
                        xc[kk][:, tq0:tq0 + 512],
                        start=(kk == 0),
                        stop=(kk == KC - 1),
                    )
                nc.vector.tensor_scalar_add(
                    dst[ft][:, tq0:tq0 + 512], ps[:], bt[ft][:]
                )

        def v_tile(xc, wv_t, bv_bc, tt):
            ps = ps_b.tile([128, 256], F32, name="psv", tag="psb")
            for kk in range(KC):
                nc.tensor.matmul(
                    ps[:],
                    xc[kk][:, tt * 128:(tt + 1) * 128],
                    wv_t[kk][:],
                    start=(kk == 0),
                    stop=(kk == KC - 1),
                )
            # bias-add into the per-head 64-wide blocks (ones col untouched)
            nc.vector.tensor_tensor(
                v_pl[tt][:, :, 0:Dh],
                ps[:].rearrange("p (h d) -> p h d", h=HPC),
                bv_bc[:, :, :],
                mybir.AluOpType.add,
            )
            nc.gpsimd.memset(v_pl[tt][:, :, Dh:Dh + 1], 1.0)

        # --- PV: per-head [65,512] PSUM banks; row 64 = softmax sum (ones
        # column of v). start=True on the first chunk owns the bank.
        def pv_chunk(pvs, p, etile, g):
            for hh in range(2):
                lh = p * 2 + hh
                sl = etile[:, hh * 512:(hh + 1) * 512]
                nc.tensor.matmul(
                    pvs[hh][:],
                    v_pl[g][:, lh, :],
                    sl,
                    start=(g == 0),
                    stop=(g == 15),
                    skip_group_check=True,
                )

        def pv_finish(pvs, p, blk):
            """Z rows staged straight from PSUM, then the big pv copies
            (releasing the banks), then recip -> broadcast -> normalize."""
            tq0 = blk * 512
            for hh in range(2):
                nc.vector.tensor_copy(zt[hh * 32:hh * 32 + 1, :], pvs[hh][64:65, :])
            pvcs = []
            for hh in range(2):
                pvc = spool.tile([64, 512], F32, name=f"pvc{hh}", tag=f"pvc{hh}")
                nc.vector.tensor_copy(pvc[:], pvs[hh][0:64, :])
                pvcs.append(pvc)
            rc = spool.tile([33, 512], F32, name="rc", tag="rc")
            nc.vector.reciprocal(rc[:], zt[:])
            # partition_broadcast always reads the tile's partition 0, so
            # stage head 1's row into a base-0 tile first
            rc1 = spool.tile([1, 512], F32, name="rc1", tag="rc1")
            nc.vector.tensor_copy(rc1[:], rc[32:33, :])
            rcaps = [rc[0:1, :], rc1[:]]
            rbs = []
            for hh in range(2):
                rb = spool.tile([64, 512], F32, name=f"rb{hh}", tag=f"rb{hh}")
                nc.gpsimd.partition_broadcast(rb[:], rcaps[hh])
                rbs.append(rb)
            for hh in range(2):
                nc.vector.tensor_tensor(
                    attnT[p][hh * 64:(hh + 1) * 64, tq0:tq0 + 512],
                    pvcs[hh][:],
                    rbs[hh][:],
                    mybir.AluOpType.mult,
                )
            return pvcs

        def wo_group(tt):
            for eb in range(2):
                ps = ps_b.tile([128, 512], F32, name="pswo", tag="psb")
                for fc in range(2):
                    nc.tensor.matmul(
                        ps[:],
                        attnT[fc][:, tt * 128:(tt + 1) * 128],
                        wo_t[fc][:, eb * 512:(eb + 1) * 512],
                        start=(fc == 0),
                        stop=(fc == 1),
                    )
                ot = opool.tile([128, 512], BF, name="ot", tag="ot")
                nc.vector.tensor_copy(ot[:], ps[:])
                nc.sync.dma_start(
                    io["out"][tt * 128:(tt + 1) * 128, eb * 512:(eb + 1) * 512],
                    ot[:],
                )

        DEFAULT_CHUNKS = {g: [2 * (g - 1), 2 * (g - 1) + 1] for g in range(1, 8)}

        def unit_emit(p, blk, fillers, n_fill, wo_pending,
                      chunk_sched=None, post_chunks=(14, 15), v_sched=None):
            """Scores+exp of (p, blk) in 8 groups of [128,1024] PSUM
            (1 tk tile x 2 heads); one 2048-wide exp per group into a
            per-group bf16 tile. PV of THIS unit runs one group behind
            by default (chunk_sched overrides, for unit 0 whose v arrives
            late). Fillers (projection blocks) are PE work with no ACT
            dependency, at most n_fill this unit; Wo groups pop 2 at
            g>=6 so the previous unit's normalize chain (DVE recip ->
            GPSIMD broadcast -> DVE mult, ~10us) never stalls the PE
            (a >3.4us PE stall makes the HAM re-throttle the clock).
            Groups with no real pad work emit a few dummy LDWEIGHTS:
            per-unit PE work (13.7us) is below the ACT exp time (18us),
            and the HAM must not see the slack as idleness."""
            tq0 = blk * 512
            pvs = [ps_pv.tile([65, 512], F32, name=f"pv{hh}", tag=f"pv{hh}")
                   for hh in range(2)]
            chunks = DEFAULT_CHUNKS if chunk_sched is None else chunk_sched
            etiles = {}
            for g in range(8):
                etile = epool.tile([128, 2 * 1024], BF, name="exp", tag="exp")
                etiles[g] = etile
                for j2 in range(2):
                    tk = g * 2 + j2
                    ps = ps_a.tile([128, 1024], F32, name="psa", tag="psa")
                    for hh in range(2):
                        pp = hh * 64
                        nc.tensor.matmul(
                            ps[:, hh * 512:(hh + 1) * 512],
                            kcT[p][pp:pp + 64, tk * 128:(tk + 1) * 128],
                            qcT[p][pp:pp + 64, tq0:tq0 + 512],
                            start=True,
                            stop=True,
                        )
                    nc.scalar.activation(
                        etile[:, j2 * 1024:(j2 + 1) * 1024],
                        ps[:],
                        mybir.ActivationFunctionType.Exp,
                        scale=1.0 / SCALE,
                    )
                acted = False
                if v_sched is not None:
                    for tt in v_sched.get(g, ()):
                        v_tile(xv, wv_t, bv_bc, tt)
                        acted = True
                if fillers and n_fill > 0:
                    fillers.pop(0)()
                    n_fill -= 1
                    acted = True
                if g >= 6 and wo_pending:
                    wo_group(wo_pending.pop(0))
                    if wo_pending:
                        wo_group(wo_pending.pop(0))
                    acted = True
                if not acted:
                    warm(4)
                for j in chunks.get(g, ()):
                    pv_chunk(pvs, p, etiles[j // 2][:, (j % 2) * 1024:(j % 2 + 1) * 1024], j)
            for j in post_chunks:
                pv_chunk(pvs, p, etiles[j // 2][:, (j % 2) * 1024:(j % 2 + 1) * 1024], j)
            return pvs

        # ================= emission schedule =================
        # DMA queue order = the order tensors are first needed; the lead-in
        # keeps the PE busy on work gated only by already-arrived tensors
        # while the rest of the 12MB of activations stream in.
        wk_t = load_w("wkT")
        bk_t = load_b("bk")
        xk = load_x("xkT")
        wq_t = load_w("wqT")
        bq_t = load_b("bq")
        xq = load_x("xqT")
        wv_t = load_w("wvT")
        bv_row = const.tile([1, F], F32, name="bvrow", tag="bvrow")
        nc.sync.dma_start(bv_row[:], io["bv"][:])
        xv = load_x("xvT")
        wo_t = []
        for fc in range(2):
            t = const.tile([128, E], BF, name=f"woT{fc}", tag=f"woT{fc}")
            nc.sync.dma_start(t[:], io["woT"][fc * 128:(fc + 1) * 128, :])
            wo_t.append(t)

        bv_full = const.tile([128, F], F32, name="bvbc", tag="bvbc")
        nc.gpsimd.partition_broadcast(bv_full[:], bv_row[:])
        bv_bc = bv_full[:].rearrange("p (h d) -> p h d", h=HPC)
        zt = const.tile([33, 512], F32, name="zt", tag="zt")
        nc.vector.memset(zt[:], 1.0)

        def warm(n):
            # dummy LDWEIGHTS: keep the PE activity monitor from
            # re-throttling the clock during unavoidable PE slack
            for _ in range(n):
                nc.tensor.ldweights(wq_t[0][:, 0:128])

        # lead-in PE work while xq/xv stream in; both stages need only xk
        proj_qk(wk_t, bk_t, kcT, xk, 0)
        proj_qk(wk_t, bk_t, kcT, xk, 1)
        proj_qk(wq_t, bq_t, qcT, xq, 0, tbps=(0,))  # needs xq (~28us)

        # remaining q projection, consumed one block per score group
        fillers = [
            lambda: proj_qk(wq_t, bq_t, qcT, xq, 0, tbps=(1,)),
            lambda: proj_qk(wq_t, bq_t, qcT, xq, 0, tbps=(2,)),
            lambda: proj_qk(wq_t, bq_t, qcT, xq, 0, tbps=(3,)),
            lambda: proj_qk(wq_t, bq_t, qcT, xq, 1, tbps=(0,)),
            lambda: proj_qk(wq_t, bq_t, qcT, xq, 1, tbps=(1,)),
            lambda: proj_qk(wq_t, bq_t, qcT, xq, 1, tbps=(2,)),
            lambda: proj_qk(wq_t, bq_t, qcT, xq, 1, tbps=(3,)),
        ]
        # unit 0: v tiles just-in-time behind the xv DMA (~41us), its PV
        # chunks late enough to follow them (epool bufs=8 keeps all the
        # unit's exp tiles live for this)
        u0_v = {5: list(range(0, 6)), 6: list(range(6, 12)), 7: list(range(12, 16))}
        u0_chunks = {6: list(range(0, 6)), 7: list(range(6, 12))}
        u0_post = tuple(range(12, 16))
        fill_budget = [3, 2, 1, 1, 0, 0, 0, 0]

        wo_pending = []
        units = [(0, 0), (0, 1), (0, 2), (0, 3), (1, 0), (1, 1), (1, 2), (1, 3)]
        for ui, (p, blk) in enumerate(units):
            if ui == 0:
                pvs = unit_emit(p, blk, fillers, 3, wo_pending,
                                chunk_sched=u0_chunks, post_chunks=u0_post,
                                v_sched=u0_v)
            else:
                pvs = unit_emit(p, blk, fillers, fill_budget[ui], wo_pending)
            pvcs = pv_finish(pvs, p, blk)
            if io["debug"]:
                for hh in range(2):
                    nc.sync.dma_start(
                        io["d_pvc"][(ui * 2 + hh) * 65:(ui * 2 + hh + 1) * 65, 0:1],
                        zt[hh * 32:hh * 32 + 1, 0:1],
                    )
                    nc.sync.dma_start(
                        io["d_pvc"][(ui * 2 + hh) * 65 + 1:(ui * 2 + hh) * 65 + 65, :],
                        pvcs[hh][:],
                    )
            if p == 1:
                wo_pending.extend(range(blk * 4, blk * 4 + 4))
        # tail: blk2's Wo is ready (normalized during unit 7); blk3 waits on
        # the last normalize chain -- keep the PE warm across that wait.
        ready = [tt for tt in wo_pending if tt < 12]
        late = [tt for tt in wo_pending if tt >= 12]
        for tt in ready:
            wo_group(tt)
        warm(36)
        for tt in late:
            wo_group(tt)

        if io["debug"]:
            for ft in range(2):
                nc.sync.dma_start(io["d_qcT"][ft * 128:(ft + 1) * 128, :], qcT[ft][:])
                nc.sync.dma_start(io["d_kcT"][ft * 128:(ft + 1) * 128, :], kcT[ft][:])
                nc.sync.dma_start(io["d_attnT"][ft * 128:(ft + 1) * 128, :], attnT[ft][:])
            for tt in range(16):
                nc.sync.dma_start(
                    io["d_vaug"][tt * 128:(tt + 1) * 128, :],
                    v_pl[tt][:].rearrange("p h d -> p (h d)"),
                )


def _build():
    nc = bacc.Bacc("TRN2", target_bir_lowering=False, debug=False)
    io = {}
    for name, shape, dt in (
        ("xqT", [E, T], BF),
        ("xkT", [E, T], BF),
        ("xvT", [E, T], BF),
        ("wqT", [E, F], BF),
        ("wkT", [E, F], BF),
        ("wvT", [E, F], BF),
        ("woT", [F, E], BF),
        ("bq", [F, 1], F32),
        ("bk", [F, 1], F32),
        ("bv", [1, F], F32),
    ):
        io[name] = nc.dram_tensor(name, shape, dt, kind="ExternalInput").ap()
    io["out"] = nc.dram_tensor("out", [T, E], BF, kind="ExternalOutput").ap()
    import os
    debug = bool(int(os.environ.get("KERNEL_DEBUG", "0")))
    if debug:
        for nm, shape in (("d_qcT", [2 * 128, T]), ("d_kcT", [2 * 128, T]),
                          ("d_attnT", [2 * 128, T]), ("d_vaug", [16 * 128, HPC * (Dh + 1)])):
            io[nm] = nc.dram_tensor(nm, shape, BF, kind="ExternalOutput").ap()
        io["d_pvc"] = nc.dram_tensor("d_pvc", [16 * 65, 512], F32,
                                     kind="ExternalOutput").ap()
    io["debug"] = debug
    with tile.TileContext(nc) as tc:
        _emit(tc, io)
    nc.compile()
    return nc


def _fold_clr(W, b, clr):
    """q_c = q - mean_head(q) + clr  ==  x @ (C W).T + (C b + clr)."""
    W64 = W.astype(np.float64).reshape(H, Dh, E)
    W_eff = W64 - W64.mean(axis=1, keepdims=True)
    b64 = b.astype(np.float64).reshape(H, Dh)
    b_eff = b64 - b64.mean(axis=1, keepdims=True) + clr.astype(np.float64).reshape(H, Dh)
    return W_eff.reshape(E, E), b_eff.reshape(E)


_NC_CACHE = None


def kernel(**inputs):
    global _NC_CACHE
    query = np.asarray(inputs["query"], np.float32)
    key = np.asarray(inputs["key"], np.float32)
    value = np.asarray(inputs["value"], np.float32)
    mask = np.asarray(inputs["key_padding_mask"])
    Wq, bq = np.asarray(inputs["Wq"], np.float32), np.asarray(inputs["bq"], np.float32)
    Wk, bk = np.asarray(inputs["Wk"], np.float32), np.asarray(inputs["bk"], np.float32)
    Wv, bv = np.asarray(inputs["Wv"], np.float32), np.asarray(inputs["bv"], np.float32)
    Wo, bo = np.asarray(inputs["Wo"], np.float32), np.asarray(inputs["bo"], np.float32)
    cq = np.asarray(inputs["clr_bias_q"], np.float32)
    ck = np.asarray(inputs["clr_bias_k"], np.float32)
    assert not mask.any(), "kernel assumes empty key_padding_mask"

    Wq_eff, bq_eff = _fold_clr(Wq, bq, cq)
    Wk_eff, bk_eff = _fold_clr(Wk, bk, ck)

    def bf(x):
        return np.ascontiguousarray(x.astype(np.float32)).astype(BF_NP)

    in_maps = []
    for c in range(NCORES):
        b = c // 4
        fs = (c % 4) * F
        m = {
            "xqT": bf(query[b].T),
            "xkT": bf(key[b].T),
            "xvT": bf(value[b].T),
            "wqT": bf(Wq_eff[fs:fs + F].T),
            "wkT": bf(Wk_eff[fs:fs + F].T),
            "wvT": bf(Wv[fs:fs + F].T),
            "woT": bf(Wo[:, fs:fs + F].T),
            "bq": np.ascontiguousarray(bq_eff[fs:fs + F, None], dtype=np.float32),
            "bk": np.ascontiguousarray(bk_eff[fs:fs + F, None], dtype=np.float32),
            "bv": np.ascontiguousarray(bv[None, fs:fs + F], dtype=np.float32),
        }
        in_maps.append(m)

    if _NC_CACHE is None:
        _NC_CACHE = _build()
    nc = _NC_CACHE

    import os

    trace = bool(int(os.environ.get("KERNEL_TRACE", "0")))
    if trace:
        _install_ntff_hook()
    res = None
    last_exc = None
    out = None
    for attempt in range(4):
        try:
            res = run_bass_kernel_spmd(
                nc, in_maps, core_ids=list(range(NCORES)), trace=trace
            )
        except Exception as e:  # transient NRT_EXEC_UNIT_UNRECOVERABLE etc.
            last_exc = e
            import time

            time.sleep(2.0)
            continue
        out = np.zeros((B, T, E), np.float32)
        for c in range(NCORES):
            out[c // 4] += res.results[c]["out"].astype(np.float32)
        if np.isfinite(out).all():
            break
        out = None  # rare transient corruption: retry
    if out is None:
        if last_exc is not None and res is None:
            raise last_exc
        raise RuntimeError("kernel produced non-finite output on all attempts")
    kernel.last_results = res
    out += bo[None, None, :].astype(np.float32)
    return out



# revision 14
# speedup vs baseline: 1.1601x; 1.1601x over previous
"""Aitchison multi-head attention on 8 trn2 NeuronCores — v2.

Per core: batch c//4, 4 heads (feature slice of 256). CLR centering is
linear -> folded into Wq/Wk + biases on host (fp64). Host sums the 4
partial output projections per batch and adds bo.

The kernel is built around the ACT-engine exp floor: 16 exps of
[128,1024] per unit (head-pair x 512-q block), ~1us each pipelined,
8 units = ~128us. Everything else is scheduled to keep that pipeline
fed from ~14us onward:
- Host pre-reshapes all tensors so every load is a simple 3D AP slice;
  input DMAs are split into tq-block pieces issued across three engine
  queues (sync/vector/gpsimd) in deadline order: xk/xq block 0 land
  ~6us in, first exp fires ~14us, xv streams in under unit 0.
- A PE warmup burst (garbage matmuls) at t=0 lifts the HAM clock gate
  (cold 1.2GHz -> 2.4GHz) before the first projection.
- scores: per group, the 2 heads' matmuls run concurrently via
  row-tiling (lhsT base partitions 0/64 -> tile_position auto).
- exp writes fp8e4; PV runs DoubleRow fp8 (contraction 256/instr, so
  PV is 2 matmuls per group instead of 4); the softmax denominator
  comes free as PSUM row 64 via a ones-column in v.
- PV chunks pop lag-1 behind their exp group from a FIFO; the 1/Z
  chain is reciprocal_approx_fast + gpsimd broadcast + DVE mult.
- k/q/v projection blocks and Wo output groups are spread across the
  units as PE filler according to their deadlines.
"""
import sys
import types

sys.path.insert(0, "/opt/trn_rl_repo")

from collections import deque

import numpy as np
import ml_dtypes

import concourse.bass as bass
import concourse.tile as tile
from concourse import bacc, mybir
from concourse.bass_utils import run_bass_kernel_spmd

B, T, E, H, Dh = 2, 2048, 1024, 16, 64
NCORES = 8
HPC = 4            # heads per core
F = HPC * Dh       # 256 features per core
SCALE = 8.0        # sqrt(Dh)
KC = E // 128      # 8 e-chunks in projections
BF = mybir.dt.bfloat16
F32 = mybir.dt.float32
FP8 = mybir.dt.float8e4
BF_NP = ml_dtypes.bfloat16

FP8_PV = False     # fp8 PV measured at rel_err 2.4e-2 (> 2e-2 gate): any
                   # fp8 in the attention path costs ~2e-2 broad noise.


def _install_ntff_hook():
    """trace=True under axon needs antenv.axon_hooks, missing in this image."""
    if "antenv.axon_hooks" in sys.modules:
        return
    try:
        from trn_agent_boot.trn_boot import _ntff_profile_via_ctypes

        hook = _ntff_profile_via_ctypes("/opt/axon/libaxon_pjrt.so")
    except Exception:
        hook = None
    mod = types.ModuleType("antenv.axon_hooks")
    mod.get_axon_ntff_profile_hook = lambda: hook
    sys.modules["antenv.axon_hooks"] = mod


def _emit(tc, io):
    nc = tc.nc
    from contextlib import ExitStack

    act_dt = FP8 if FP8_PV else BF

    ctx = ExitStack()
    with ctx:
        const = ctx.enter_context(tc.tile_pool(name="const", bufs=1))
        qk = ctx.enter_context(tc.tile_pool(name="qk", bufs=1))
        epool = ctx.enter_context(tc.tile_pool(name="exp", bufs=9))
        spool = ctx.enter_context(tc.tile_pool(name="small", bufs=2))
        opool = ctx.enter_context(tc.tile_pool(name="out", bufs=2))
        ps_a = ctx.enter_context(tc.tile_pool(name="psa", bufs=2, space="PSUM"))
        ps_pv = ctx.enter_context(tc.tile_pool(name="pspv", bufs=1, space="PSUM"))
        ps_b = ctx.enter_context(tc.tile_pool(name="psb", bufs=2, space="PSUM"))

        # ---------------- persistent tiles ----------------
        wk = const.tile([128, KC, F], BF, name="wk", tag="wk")
        wq = const.tile([128, KC, F], BF, name="wq", tag="wq")
        wv = const.tile([128, KC, F], BF, name="wv", tag="wv")
        wo_t = const.tile([128, 2, E], BF, name="wo", tag="wo")
        bk_t = const.tile([128, 2], F32, name="bk", tag="bk")
        bq_t = const.tile([128, 2], F32, name="bq", tag="bq")
        bv_row = const.tile([1, F], F32, name="bvrow", tag="bvrow")
        scratch = const.tile([128, 1024], BF, name="scr", tag="scr")

        qcT = [qk.tile([128, T], BF, name=f"qcT{ft}", tag=f"qcT{ft}") for ft in range(2)]
        kcT = [qk.tile([128, T], BF, name=f"kcT{ft}", tag=f"kcT{ft}") for ft in range(2)]
        attnT = [qk.tile([128, T], BF, name=f"attnT{ft}", tag=f"attnT{ft}") for ft in range(2)]
        # v, augmented with a ones column per head, as 8 super-chunks of
        # 256 tk each: [tk_lo 128, plane 2, head 4, 68(64 v | 1 one | pad)]
        v2 = [const.tile([128, 2, HPC, 68], act_dt, name=f"v2_{G}", tag=f"v2_{G}")
              for G in range(8)]

        # x inputs as [128, chunk, 512] block tiles; xb[(which, b, kk)] -> AP
        xb = {}

        def alloc_x(which, b, cs):
            t = const.tile([128, len(cs), 512], BF, name=f"x{which}{b}",
                           tag=f"x{which}_{b}_{cs[0]}")
            for i, kk in enumerate(cs):
                xb[(which, b, kk)] = t[:, i, :]
            return t

        # ---------------- DMA issue (deadline order, 3 queues) ----------------
        # memset scratch first so warmup matmuls read defined data
        nc.vector.memset(scratch[:], 1.0)

        def dma_x(eng, which, b, cs):
            t = alloc_x(which, b, cs)
            eng.dma_start(t[:], io["x" + which][:, cs[0]:cs[0] + len(cs),
                                                b * 512:(b + 1) * 512])

        # sync + scalar are HW-DGE queues (fast); gpsimd is SW-DGE (tiny
        # loads only). scalar is used for 8 early issues only, so the ACT
        # table load still lands well before exp #0. The link runs at
        # ~358GB/s aggregate once flowing; arrival order == issue order,
        # so issues are sorted by consumer deadline.
        sy, sc, gp = nc.sync, nc.scalar, nc.gpsimd
        # wave A: everything the lead-in needs (~3MB -> done ~20us)
        sc.dma_start(wk[:, 0:4, :], io["wk"][:, 0:4, :])
        sc.dma_start(wk[:, 4:8, :], io["wk"][:, 4:8, :])
        dma_x(sy, "k", 0, [0, 1]); dma_x(sy, "k", 0, [2, 3])
        dma_x(sc, "k", 0, [4, 5]); dma_x(sc, "k", 0, [6, 7])
        sc.dma_start(wq[:, 0:4, :], io["wq"][:, 0:4, :])
        sc.dma_start(wq[:, 4:8, :], io["wq"][:, 4:8, :])
        dma_x(sy, "q", 0, [0, 1]); dma_x(sy, "q", 0, [2, 3])
        dma_x(sc, "q", 0, [4, 5]); dma_x(sc, "q", 0, [6, 7])
        gp.dma_start(bk_t[:], io["bk"][:, :])
        gp.dma_start(bq_t[:], io["bq"][:, :])
        # rest of k first (kcT blocks are consumed by unit-0 score groups),
        # then v blocks (vproj under units 0-1), xq_b1 wedged in before
        # xv_b2 so unit 1's q block makes its deadline.
        dma_x(sy, "k", 1, [0, 1, 2, 3]); dma_x(sy, "k", 1, [4, 5, 6, 7])
        dma_x(sy, "k", 2, [0, 1, 2, 3]); dma_x(sy, "k", 2, [4, 5, 6, 7])
        dma_x(sy, "k", 3, [0, 1, 2, 3]); dma_x(sy, "k", 3, [4, 5, 6, 7])
        sy.dma_start(wv[:, 0:4, :], io["wv"][:, 0:4, :])
        sy.dma_start(wv[:, 4:8, :], io["wv"][:, 4:8, :])
        dma_x(sy, "v", 0, [0, 1, 2, 3]); dma_x(sy, "v", 0, [4, 5, 6, 7])
        gp.dma_start(bv_row[:], io["bv"][:, :])
        dma_x(sy, "v", 1, [0, 1, 2, 3]); dma_x(sy, "v", 1, [4, 5, 6, 7])
        dma_x(sy, "q", 1, [0, 1, 2, 3]); dma_x(sy, "q", 1, [4, 5, 6, 7])
        dma_x(sy, "v", 2, [0, 1, 2, 3]); dma_x(sy, "v", 2, [4, 5, 6, 7])
        dma_x(sy, "v", 3, [0, 1, 2, 3]); dma_x(sy, "v", 3, [4, 5, 6, 7])
        dma_x(sy, "q", 2, [0, 1, 2, 3]); dma_x(sy, "q", 2, [4, 5, 6, 7])
        dma_x(sy, "q", 3, [0, 1, 2, 3]); dma_x(sy, "q", 3, [4, 5, 6, 7])
        sy.dma_start(wo_t[:, 0, :], io["wo"][:, 0, :])
        sy.dma_start(wo_t[:, 1, :], io["wo"][:, 1, :])

        # broadcast bv across partitions: [128, 4, 64] f32
        bv_full = const.tile([128, F], F32, name="bvbc", tag="bvbc")
        nc.gpsimd.partition_broadcast(bv_full[:], bv_row[:])
        bv_bc = bv_full[:].rearrange("p (h d) -> p h d", h=HPC)
        zt = const.tile([33, 512], F32, name="zt", tag="zt")
        nc.vector.memset(zt[:], 1.0)
        # select matrix for the 1/Z partition-broadcast outer product:
        # rb[128,512] = sel.T @ rc puts rc row 0 on partitions 0:64 and
        # rc row 32 on partitions 64:128 (one PE matmul, no gpsimd).
        sel = const.tile([33, 128], F32, name="sel", tag="sel")
        nc.vector.memset(sel[:], 0.0)
        nc.vector.memset(sel[0:1, 0:64], 1.0)
        nc.vector.memset(sel[32:33, 64:128], 1.0)

        # ---------------- compute building blocks ----------------
        def warmup(n):
            # garbage matmuls: lift the HAM clock gate during the DMA wait
            for _ in range(n):
                ps = ps_b.tile([128, 512], F32, name="warm", tag="psb")
                nc.tensor.matmul(ps[:], scratch[:, 0:128], scratch[:, 0:512],
                                 start=True, stop=True)

        def proj_block(which, ft, b):
            wt, bt, dst = ((wk, bk_t, kcT) if which == "k" else (wq, bq_t, qcT))
            ps = ps_b.tile([128, 512], F32, name="psp", tag="psb")
            for kk in range(KC):
                nc.tensor.matmul(ps[:], wt[:, kk, ft * 128:(ft + 1) * 128],
                                 xb[(which, b, kk)],
                                 start=(kk == 0), stop=(kk == KC - 1))
            nc.vector.tensor_scalar_add(
                dst[ft][:, b * 512:(b + 1) * 512], ps[:], bt[:, ft:ft + 1])

        def v_tile(tt):
            G, j = tt // 2, tt % 2
            ps = ps_b.tile([128, 256], F32, name="psv", tag="psb")
            for kk in range(KC):
                nc.tensor.matmul(ps[:],
                                 xb[("v", tt // 4, kk)][:, (tt % 4) * 128:(tt % 4 + 1) * 128],
                                 wv[:, kk, :],
                                 start=(kk == 0), stop=(kk == KC - 1))
            nc.vector.tensor_tensor(
                v2[G][:, j, :, 0:Dh],
                ps[:].rearrange("p (h d) -> p h d", h=HPC),
                bv_bc[:, :, :],
                mybir.AluOpType.add)
            nc.gpsimd.memset(v2[G][:, j, :, Dh:Dh + 1], 1.0)

        units = [(0, 0), (0, 1), (0, 2), (0, 3), (1, 0), (1, 1), (1, 2), (1, 3)]
        pvs_tiles = {}
        etiles = {}

        def pv_chunk(u, G):
            p, blk = units[u]
            if u not in pvs_tiles:
                pvs_tiles[u] = [ps_pv.tile([65, 512], F32, name=f"pv{hh}", tag=f"pv{hh}")
                                for hh in range(2)]
            pvs = pvs_tiles[u]
            et = etiles[(u, G)][:].rearrange("p (j q) -> p j q", j=2)
            for hh in range(2):
                lh = p * 2 + hh
                rhs = et[:, :, hh * 512:(hh + 1) * 512]      # [128, 2, 512]
                if FP8_PV:
                    nc.tensor.matmul(pvs[hh][:], v2[G][:, :, lh, 0:Dh + 1], rhs,
                                     start=(G == 0), stop=(G == 7),
                                     perf_mode=mybir.MatmulPerfMode.DoubleRow,
                                     skip_group_check=True)
                else:
                    for j in range(2):
                        nc.tensor.matmul(pvs[hh][:], v2[G][:, j, lh, 0:Dh + 1],
                                         rhs[:, j, :],
                                         start=(G == 0 and j == 0),
                                         stop=(G == 7 and j == 1),
                                         skip_group_check=True)

        def chain(u):
            """1/Z normalize for unit u's PV banks -> attnT (bf16)."""
            p, blk = units[u]
            tq0 = blk * 512
            pvs = pvs_tiles[u]
            for hh in range(2):
                nc.vector.tensor_copy(zt[hh * 32:hh * 32 + 1, :], pvs[hh][64:65, :])
            pvcs = []
            for hh in range(2):
                pvc = spool.tile([64, 512], BF, name=f"pvc{hh}", tag=f"pvc{hh}")
                nc.vector.tensor_copy(pvc[:], pvs[hh][0:64, :])
                pvcs.append(pvc)
            rc = spool.tile([33, 512], F32, name="rc", tag="rc")
            nc.vector.reciprocal_approx_fast(rc[:], zt[:])
            rb = ps_b.tile([128, 512], F32, name="rb", tag="psb")
            nc.tensor.matmul(rb[:], sel[:], rc[:], start=True, stop=True)
            for hh in range(2):
                nc.vector.tensor_tensor(
                    attnT[p][hh * 64:(hh + 1) * 64, tq0:tq0 + 512],
                    pvcs[hh][:], rb[hh * 64:(hh + 1) * 64, :],
                    mybir.AluOpType.mult)

        def wo_group(tt, split_dma=False):
            ot = opool.tile([128, E], BF, name="ot", tag="ot")
            for eb in range(2):
                ps = ps_b.tile([128, 512], F32, name="pswo", tag="psb")
                for fc in range(2):
                    nc.tensor.matmul(ps[:], attnT[fc][:, tt * 128:(tt + 1) * 128],
                                     wo_t[:, fc, eb * 512:(eb + 1) * 512],
                                     start=(fc == 0), stop=(fc == 1))
                nc.vector.tensor_copy(ot[:, eb * 512:(eb + 1) * 512], ps[:])
                if split_dma:
                    # tail: overlap the two half-row DMAs on both HW queues
                    eng = nc.sync if eb == 0 else nc.scalar
                    eng.dma_start(
                        io["out"][tt * 128:(tt + 1) * 128, eb * 512:(eb + 1) * 512],
                        ot[:, eb * 512:(eb + 1) * 512])
            if not split_dma:
                nc.sync.dma_start(io["out"][tt * 128:(tt + 1) * 128, :], ot[:])

        # ---------------- schedule ----------------
        # fillers placed by deadline vs DMA arrival (~358GB/s in issue
        # order); (ui, g) -> list of closures
        sched = {
            (0, 0): [lambda: proj_block("k", 0, 1)],
            (0, 2): [lambda: proj_block("k", 0, 2)],
            (0, 4): [lambda: proj_block("k", 0, 3)],
            (0, 6): [lambda: v_tile(0), lambda: v_tile(1)],
            (0, 7): [lambda: v_tile(2), lambda: v_tile(3), lambda: proj_block("q", 0, 1)],
            (1, 0): [lambda: v_tile(4), lambda: v_tile(5)],
            (1, 1): [lambda: v_tile(6), lambda: v_tile(7)],
            (1, 2): [lambda: v_tile(8), lambda: v_tile(9)],
            (1, 3): [lambda: v_tile(10), lambda: v_tile(11)],
            (1, 4): [lambda: v_tile(12), lambda: v_tile(13)],
            (1, 5): [lambda: v_tile(14), lambda: v_tile(15)],
            (1, 7): [lambda: proj_block("q", 0, 2)],
            (2, 1): [lambda: proj_block("k", 1, 0)],
            (2, 3): [lambda: proj_block("k", 1, 1)],
            (2, 4): [lambda: proj_block("q", 0, 3)],
            (2, 5): [lambda: proj_block("k", 1, 2)],
            (2, 7): [lambda: proj_block("k", 1, 3)],
            (3, 1): [lambda: proj_block("q", 1, 0)],
            (3, 3): [lambda: proj_block("q", 1, 1)],
            (4, 1): [lambda: proj_block("q", 1, 2)],
            (5, 1): [lambda: proj_block("q", 1, 3), lambda: wo_group(0)],
            (5, 3): [lambda: wo_group(1)],
            (5, 5): [lambda: wo_group(2)],
            (5, 7): [lambda: wo_group(3)],
            (6, 1): [lambda: wo_group(4)],
            (6, 3): [lambda: wo_group(5)],
            (6, 5): [lambda: wo_group(6)],
            (6, 7): [lambda: wo_group(7)],
            (7, 1): [lambda: wo_group(8)],
            (7, 3): [lambda: wo_group(9)],
            (7, 5): [lambda: wo_group(10)],
            (7, 7): [lambda: wo_group(11)],
        }

        # PV FIFO: chunk (u, G) may emit once exp (u, G+1) is emitted
        # (lag-1) and v2[G] is fully projected; chain(u) follows chunk(u,7).
        pv_q = deque()
        emitted = set()
        vcount = [0]

        def pv_ready(item, ui, g):
            kind = item[0]
            if kind == "chain":
                return True
            _, u, G = item
            if vcount[0] < 2 * (G + 1):
                return False
            need = (u, G + 1) if G < 7 else ((u + 1, 0) if u < 7 else None)
            return need is None or need in emitted

        def drain_pv(ui, g, cap=3):
            n = 0
            while pv_q and n < cap:
                item = pv_q[0]
                if not pv_ready(item, ui, g):
                    break
                pv_q.popleft()
                if item[0] == "chain":
                    chain(item[1])
                else:
                    pv_chunk(item[1], item[2])
                    n += 1

        # ---------------- emission ----------------
        warmup(14)
        proj_block("k", 0, 0)
        proj_block("q", 0, 0)
        proj_block("k", 0, 1)

        orig_vtile = v_tile

        def v_tile_counted(tt):
            orig_vtile(tt)
            vcount[0] += 1

        v_tile = v_tile_counted
        # patch sched closures to use counted v_tile: rebuild lazily instead
        # (closures above captured the name `v_tile` at call time in this
        # scope, so they already see the counted version)

        for ui, (p, blk) in enumerate(units):
            for G in range(8):
                pv_q.append(("chunk", ui, G))
            pv_q.append(("chain", ui))
            for g in range(8):
                etile = epool.tile([128, 2 * 1024], act_dt, name="exp", tag="exp")
                etiles[(ui, g)] = etile
                for j2 in range(2):
                    tk = g * 2 + j2
                    ps = ps_a.tile([128, 1024], F32, name="psa", tag="psa")
                    for hh in range(2):
                        pp = hh * 64
                        nc.tensor.matmul(
                            ps[:, hh * 512:(hh + 1) * 512],
                            kcT[p][pp:pp + 64, tk * 128:(tk + 1) * 128],
                            qcT[p][pp:pp + 64, blk * 512:blk * 512 + 512],
                            start=True, stop=True)
                    nc.scalar.activation(
                        etile[:, j2 * 1024:(j2 + 1) * 1024], ps[:],
                        mybir.ActivationFunctionType.Exp, scale=1.0 / SCALE)
                emitted.add((ui, g))
                drain_pv(ui, g)
                for fn in sched.get((ui, g), ()):
                    fn()
        # tail: flush remaining PV chunks + chain, then last Wo block
        while pv_q:
            item = pv_q.popleft()
            if item[0] == "chain":
                chain(item[1])
            else:
                pv_chunk(item[1], item[2])
        for tt in (12, 13, 14, 15):
            wo_group(tt, split_dma=True)


def _build():
    nc = bacc.Bacc("TRN2", target_bir_lowering=False, debug=False)
    io = {}
    for name, shape, dt in (
        ("xq", [128, KC, T], BF),
        ("xk", [128, KC, T], BF),
        ("xv", [128, KC, T], BF),
        ("wq", [128, KC, F], BF),
        ("wk", [128, KC, F], BF),
        ("wv", [128, KC, F], BF),
        ("wo", [128, 2, E], BF),
        ("bq", [128, 2], F32),
        ("bk", [128, 2], F32),
        ("bv", [1, F], F32),
    ):
        io[name] = nc.dram_tensor(name, shape, dt, kind="ExternalInput").ap()
    io["out"] = nc.dram_tensor("out", [T, E], BF, kind="ExternalOutput").ap()
    with tile.TileContext(nc) as tc:
        _emit(tc, io)
    nc.compile()
    return nc


def _fold_clr(W, b, clr):
    """q_c = q - mean_head(q) + clr  ==  x @ (C W).T + (C b + clr)."""
    W64 = W.astype(np.float64).reshape(H, Dh, E)
    W_eff = W64 - W64.mean(axis=1, keepdims=True)
    b64 = b.astype(np.float64).reshape(H, Dh)
    b_eff = b64 - b64.mean(axis=1, keepdims=True) + clr.astype(np.float64).reshape(H, Dh)
    return W_eff.reshape(E, E), b_eff.reshape(E)


_NC_CACHE = None


def _chunk3(a, nchunk):
    """[nchunk*128, M] -> [128, nchunk, M]"""
    n, m = a.shape
    return np.ascontiguousarray(
        a.reshape(nchunk, 128, m).transpose(1, 0, 2))


def kernel(**inputs):
    global _NC_CACHE
    query = np.asarray(inputs["query"], np.float32)
    key = np.asarray(inputs["key"], np.float32)
    value = np.asarray(inputs["value"], np.float32)
    mask = np.asarray(inputs["key_padding_mask"])
    Wq, bq = np.asarray(inputs["Wq"], np.float32), np.asarray(inputs["bq"], np.float32)
    Wk, bk = np.asarray(inputs["Wk"], np.float32), np.asarray(inputs["bk"], np.float32)
    Wv, bv = np.asarray(inputs["Wv"], np.float32), np.asarray(inputs["bv"], np.float32)
    Wo, bo = np.asarray(inputs["Wo"], np.float32), np.asarray(inputs["bo"], np.float32)
    cq = np.asarray(inputs["clr_bias_q"], np.float32)
    ck = np.asarray(inputs["clr_bias_k"], np.float32)
    assert not mask.any(), "kernel assumes empty key_padding_mask"

    Wq_eff, bq_eff = _fold_clr(Wq, bq, cq)
    Wk_eff, bk_eff = _fold_clr(Wk, bk, ck)

    def bf(x):
        return np.ascontiguousarray(x.astype(np.float32)).astype(BF_NP)

    in_maps = []
    for c in range(NCORES):
        b = c // 4
        fs = (c % 4) * F
        m = {
            "xq": bf(_chunk3(query[b].T, KC)),
            "xk": bf(_chunk3(key[b].T, KC)),
            "xv": bf(_chunk3(value[b].T, KC)),
            "wq": bf(_chunk3(Wq_eff[fs:fs + F].T, KC)),
            "wk": bf(_chunk3(Wk_eff[fs:fs + F].T, KC)),
            "wv": bf(_chunk3(Wv[fs:fs + F].T, KC)),
            "wo": bf(_chunk3(Wo[:, fs:fs + F].T, 2)),
            "bq": np.ascontiguousarray(
                bq_eff[fs:fs + F].reshape(2, 128).T.astype(np.float32)),
            "bk": np.ascontiguousarray(
                bk_eff[fs:fs + F].reshape(2, 128).T.astype(np.float32)),
            "bv": np.ascontiguousarray(bv[None, fs:fs + F], dtype=np.float32),
        }
        in_maps.append(m)

    if _NC_CACHE is None:
        _NC_CACHE = _build()
    nc = _NC_CACHE

    import os

    trace = bool(int(os.environ.get("KERNEL_TRACE", "0")))
    if trace:
        _install_ntff_hook()
    res = None
    last_exc = None
    out = None
    for attempt in range(4):
        try:
            res = run_bass_kernel_spmd(
                nc, in_maps, core_ids=list(range(NCORES)), trace=trace
            )
        except Exception as e:  # transient NRT_EXEC_UNIT_UNRECOVERABLE etc.
            last_exc = e
            import time

            time.sleep(2.0)
            continue
        out = np.zeros((B, T, E), np.float32)
        for c in range(NCORES):
            out[c // 4] += res.results[c]["out"].astype(np.float32)
        if np.isfinite(out).all():
            break
        out = None  # rare transient corruption: retry
    if out is None:
        if last_exc is not None and res is None:
            raise last_exc
        raise RuntimeError("kernel produced non-finite output on all attempts")
    kernel.last_results = res
    out += bo[None, None, :].astype(np.float32)
    return out


# revision 27
# speedup vs baseline: 1.2281x; 1.0587x over previous
"""Aitchison multi-head attention on 8 trn2 NeuronCores — v2.

Per core: batch c//4, 4 heads (feature slice of 256). CLR centering is
linear -> folded into Wq/Wk + biases on host (fp64). Host sums the 4
partial output projections per batch and adds bo.

The kernel is built around the ACT-engine exp floor: 16 exps of
[128,1024] per unit (head-pair x 512-q block), ~1us each pipelined,
8 units = ~128us. Everything else is scheduled to keep that pipeline
fed from ~14us onward:
- Host pre-reshapes all tensors so every load is a simple 3D AP slice;
  input DMAs are split into tq-block pieces issued across three engine
  queues (sync/vector/gpsimd) in deadline order: xk/xq block 0 land
  ~6us in, first exp fires ~14us, xv streams in under unit 0.
- A PE warmup burst (garbage matmuls) at t=0 lifts the HAM clock gate
  (cold 1.2GHz -> 2.4GHz) before the first projection.
- scores: per group, the 2 heads' matmuls run concurrently via
  row-tiling (lhsT base partitions 0/64 -> tile_position auto).
- exp writes fp8e4; PV runs DoubleRow fp8 (contraction 256/instr, so
  PV is 2 matmuls per group instead of 4); the softmax denominator
  comes free as PSUM row 64 via a ones-column in v.
- PV chunks pop lag-1 behind their exp group from a FIFO; the 1/Z
  chain is reciprocal_approx_fast + gpsimd broadcast + DVE mult.
- k/q/v projection blocks and Wo output groups are spread across the
  units as PE filler according to their deadlines.
"""
import sys
import types

sys.path.insert(0, "/opt/trn_rl_repo")

from collections import deque

import numpy as np
import ml_dtypes

import concourse.bass as bass
import concourse.tile as tile
from concourse import bacc, mybir
from concourse.bass_utils import run_bass_kernel_spmd

B, T, E, H, Dh = 2, 2048, 1024, 16, 64
NCORES = 8
HPC = 4            # heads per core
F = HPC * Dh       # 256 features per core
SCALE = 8.0        # sqrt(Dh)
KC = E // 128      # 8 e-chunks in projections
BF = mybir.dt.bfloat16
F32 = mybir.dt.float32
FP8 = mybir.dt.float8e4
BF_NP = ml_dtypes.bfloat16

FP8_PV = False     # fp8 PV measured at rel_err 2.4e-2 (> 2e-2 gate): any
                   # fp8 in the attention path costs ~2e-2 broad noise.


def _install_ntff_hook():
    """trace=True under axon needs antenv.axon_hooks, missing in this image."""
    if "antenv.axon_hooks" in sys.modules:
        return
    try:
        from trn_agent_boot.trn_boot import _ntff_profile_via_ctypes

        hook = _ntff_profile_via_ctypes("/opt/axon/libaxon_pjrt.so")
    except Exception:
        hook = None
    mod = types.ModuleType("antenv.axon_hooks")
    mod.get_axon_ntff_profile_hook = lambda: hook
    sys.modules["antenv.axon_hooks"] = mod


def _emit(tc, io):
    nc = tc.nc
    from contextlib import ExitStack

    act_dt = FP8 if FP8_PV else BF

    ctx = ExitStack()
    with ctx:
        const = ctx.enter_context(tc.tile_pool(name="const", bufs=1))
        qk = ctx.enter_context(tc.tile_pool(name="qk", bufs=1))
        epool = ctx.enter_context(tc.tile_pool(name="exp", bufs=8))
        spool = ctx.enter_context(tc.tile_pool(name="small", bufs=2))
        opool = ctx.enter_context(tc.tile_pool(name="out", bufs=2))
        ps_a = ctx.enter_context(tc.tile_pool(name="psa", bufs=2, space="PSUM"))
        ps_pv = ctx.enter_context(tc.tile_pool(name="pspv", bufs=1, space="PSUM"))
        ps_b = ctx.enter_context(tc.tile_pool(name="psb", bufs=2, space="PSUM"))

        # ---------------- persistent tiles ----------------
        wk = const.tile([128, KC, F], BF, name="wk", tag="wk")
        wq = const.tile([128, KC, F], BF, name="wq", tag="wq")
        wv = const.tile([128, KC, F], BF, name="wv", tag="wv")
        wo_t = const.tile([128, 2, E], BF, name="wo", tag="wo")
        bk_t = const.tile([128, 2], F32, name="bk", tag="bk")
        bq_t = const.tile([128, 2], F32, name="bq", tag="bq")
        bv_row = const.tile([1, F], F32, name="bvrow", tag="bvrow")
        scratch = const.tile([128, 512], BF, name="scr", tag="scr")

        qcT = [qk.tile([128, T], BF, name=f"qcT{ft}", tag=f"qcT{ft}") for ft in range(2)]
        kcT = [qk.tile([128, T], BF, name=f"kcT{ft}", tag=f"kcT{ft}") for ft in range(2)]
        attnT = [qk.tile([128, T], BF, name=f"attnT{ft}", tag=f"attnT{ft}") for ft in range(2)]
        # v, augmented with a ones column per head, as 8 super-chunks of
        # 256 tk each: [tk_lo 128, plane 2, head 4, 68(64 v | 1 one | pad)]
        v2 = [const.tile([128, 2, HPC, 68], act_dt, name=f"v2_{G}", tag=f"v2_{G}")
              for G in range(8)]

        # x inputs as [128, chunk, 512] block tiles; xb[(which, b, kk)] -> AP
        xb = {}

        def alloc_x(which, b, cs):
            t = const.tile([128, len(cs), 512], BF, name=f"x{which}{b}",
                           tag=f"x{which}_{b}_{cs[0]}")
            for i, kk in enumerate(cs):
                xb[(which, b, kk)] = t[:, i, :]
            return t

        # ---------------- DMA issue (deadline order, 3 queues) ----------------
        # memset scratch first so warmup matmuls read defined data
        nc.vector.memset(scratch[:], 1.0)

        def dma_x(eng, which, b, cs):
            t = alloc_x(which, b, cs)
            eng.dma_start(t[:], io["x" + which][:, cs[0]:cs[0] + len(cs),
                                                b * 512:(b + 1) * 512])

        # sync + scalar are HW-DGE queues (fast); gpsimd is SW-DGE (tiny
        # loads only). scalar is used for 8 early issues only, so the ACT
        # table load still lands well before exp #0. The link runs at
        # ~358GB/s aggregate once flowing; arrival order == issue order,
        # so issues are sorted by consumer deadline.
        # only the two HW-DGE queues (sync + scalar); no gpsimd SW-DGE ring
        sy, sc = nc.sync, nc.scalar
        # wave A: everything the lead-in needs (~3MB -> done ~20us)
        sc.dma_start(wk[:, 0:4, :], io["wk"][:, 0:4, :])
        sc.dma_start(wk[:, 4:8, :], io["wk"][:, 4:8, :])
        sc.dma_start(bk_t[:], io["bk"][:, :])
        sc.dma_start(bq_t[:], io["bq"][:, :])
        dma_x(sy, "k", 0, [0, 1]); dma_x(sy, "k", 0, [2, 3])
        dma_x(sc, "k", 0, [4, 5]); dma_x(sc, "k", 0, [6, 7])
        sc.dma_start(wq[:, 0:4, :], io["wq"][:, 0:4, :])
        sc.dma_start(wq[:, 4:8, :], io["wq"][:, 4:8, :])
        dma_x(sy, "q", 0, [0, 1]); dma_x(sy, "q", 0, [2, 3])
        dma_x(sc, "q", 0, [4, 5]); dma_x(sc, "q", 0, [6, 7])
        sc.dma_start(bv_row[:], io["bv"][:, :])
        # rest of k first (kcT blocks are consumed by unit-0 score groups),
        # then v blocks (vproj under units 0-1), xq_b1 wedged in before
        # xv_b2 so unit 1's q block makes its deadline.
        dma_x(sy, "k", 1, [0, 1, 2, 3]); dma_x(sy, "k", 1, [4, 5, 6, 7])
        dma_x(sy, "k", 2, [0, 1, 2, 3]); dma_x(sy, "k", 2, [4, 5, 6, 7])
        dma_x(sy, "k", 3, [0, 1, 2, 3]); dma_x(sy, "k", 3, [4, 5, 6, 7])
        sy.dma_start(wv[:, 0:4, :], io["wv"][:, 0:4, :])
        sy.dma_start(wv[:, 4:8, :], io["wv"][:, 4:8, :])
        dma_x(sy, "v", 0, [0, 1, 2, 3]); dma_x(sy, "v", 0, [4, 5, 6, 7])
        dma_x(sy, "v", 1, [0, 1, 2, 3]); dma_x(sy, "v", 1, [4, 5, 6, 7])
        dma_x(sy, "q", 1, [0, 1, 2, 3]); dma_x(sy, "q", 1, [4, 5, 6, 7])
        dma_x(sy, "v", 2, [0, 1, 2, 3]); dma_x(sy, "v", 2, [4, 5, 6, 7])
        dma_x(sy, "v", 3, [0, 1, 2, 3]); dma_x(sy, "v", 3, [4, 5, 6, 7])
        dma_x(sy, "q", 2, [0, 1, 2, 3]); dma_x(sy, "q", 2, [4, 5, 6, 7])
        dma_x(sy, "q", 3, [0, 1, 2, 3]); dma_x(sy, "q", 3, [4, 5, 6, 7])
        sy.dma_start(wo_t[:, 0, :], io["wo"][:, 0, :])
        sy.dma_start(wo_t[:, 1, :], io["wo"][:, 1, :])

        # broadcast bv across partitions: [128, 4, 64] f32
        bv_full = const.tile([128, F], F32, name="bvbc", tag="bvbc")
        nc.gpsimd.partition_broadcast(bv_full[:], bv_row[:])
        bv_bc = bv_full[:].rearrange("p (h d) -> p h d", h=HPC)
        zt = const.tile([33, 512], F32, name="zt", tag="zt")
        nc.vector.memset(zt[:], 1.0)

        # ---------------- compute building blocks ----------------
        def warmup(n):
            # garbage matmuls: lift/hold the HAM clock gate during DMA
            # waits. Allocated from ps_a (unused while no unit is live /
            # rotation-safe) so they never sit inside an open ps_b group.
            for _ in range(n):
                ps = ps_a.tile([128, 1024], F32, name="warm", tag="psa")
                nc.tensor.matmul(ps[:, 0:512], scratch[:, 0:128],
                                 scratch[:, 0:512], start=True, stop=True)

        def proj_block(which, ft, b, pace=0):
            # pace>0: interleave warmup matmuls between the DMA-gated
            # chunk matmuls so the PE never idles during the lead-in.
            wt, bt, dst = ((wk, bk_t, kcT) if which == "k" else (wq, bq_t, qcT))
            ps = ps_b.tile([128, 512], F32, name="psp", tag="psb")
            for kk in range(KC):
                if pace and kk < 6:
                    warmup(pace)
                nc.tensor.matmul(ps[:], wt[:, kk, ft * 128:(ft + 1) * 128],
                                 xb[(which, b, kk)],
                                 start=(kk == 0), stop=(kk == KC - 1))
            nc.vector.tensor_scalar_add(
                dst[ft][:, b * 512:(b + 1) * 512], ps[:], bt[:, ft:ft + 1])

        def v_tile(tt):
            G, j = tt // 2, tt % 2
            ps = ps_b.tile([128, 256], F32, name="psv", tag="psb")
            for kk in range(KC):
                nc.tensor.matmul(ps[:],
                                 xb[("v", tt // 4, kk)][:, (tt % 4) * 128:(tt % 4 + 1) * 128],
                                 wv[:, kk, :],
                                 start=(kk == 0), stop=(kk == KC - 1))
            nc.vector.tensor_tensor(
                v2[G][:, j, :, 0:Dh],
                ps[:].rearrange("p (h d) -> p h d", h=HPC),
                bv_bc[:, :, :],
                mybir.AluOpType.add)
            nc.gpsimd.memset(v2[G][:, j, :, Dh:Dh + 1], 1.0)

        units = [(0, 0), (0, 1), (0, 2), (0, 3), (1, 0), (1, 1), (1, 2), (1, 3)]
        pvs_tiles = {}
        etiles = {}

        def pv_chunk(u, G):
            p, blk = units[u]
            if u not in pvs_tiles:
                pvs_tiles[u] = [ps_pv.tile([65, 512], F32, name=f"pv{hh}", tag=f"pv{hh}")
                                for hh in range(2)]
            pvs = pvs_tiles[u]
            et = etiles[(u, G)][:].rearrange("p (j q) -> p j q", j=2)
            for hh in range(2):
                lh = p * 2 + hh
                rhs = et[:, :, hh * 512:(hh + 1) * 512]      # [128, 2, 512]
                if FP8_PV:
                    nc.tensor.matmul(pvs[hh][:], v2[G][:, :, lh, 0:Dh + 1], rhs,
                                     start=(G == 0), stop=(G == 7),
                                     perf_mode=mybir.MatmulPerfMode.DoubleRow,
                                     skip_group_check=True)
                else:
                    for j in range(2):
                        nc.tensor.matmul(pvs[hh][:], v2[G][:, j, lh, 0:Dh + 1],
                                         rhs[:, j, :],
                                         start=(G == 0 and j == 0),
                                         stop=(G == 7 and j == 1),
                                         skip_group_check=True)

        def chain(u):
            """1/Z normalize for unit u's PV banks -> attnT (bf16)."""
            p, blk = units[u]
            tq0 = blk * 512
            pvs = pvs_tiles[u]
            for hh in range(2):
                nc.vector.tensor_copy(zt[hh * 32:hh * 32 + 1, :], pvs[hh][64:65, :])
            pvcs = []
            for hh in range(2):
                pvc = spool.tile([64, 512], BF, name=f"pvc{hh}", tag=f"pvc{hh}")
                nc.vector.tensor_copy(pvc[:], pvs[hh][0:64, :])
                pvcs.append(pvc)
            rc = spool.tile([33, 512], F32, name="rc", tag="rc")
            nc.vector.reciprocal_approx_fast(rc[:], zt[:])
            rc1 = spool.tile([1, 512], F32, name="rc1", tag="rc1")
            nc.vector.tensor_copy(rc1[:], rc[32:33, :])
            rcaps = [rc[0:1, :], rc1[:]]
            for hh in range(2):
                rb = spool.tile([64, 512], F32, name=f"rb{hh}", tag=f"rb{hh}")
                nc.gpsimd.partition_broadcast(rb[:], rcaps[hh])
                nc.vector.tensor_tensor(
                    attnT[p][hh * 64:(hh + 1) * 64, tq0:tq0 + 512],
                    pvcs[hh][:], rb[:], mybir.AluOpType.mult)

        def wo_group(tt, split_dma=False):
            ot = opool.tile([128, E], BF, name="ot", tag="ot")
            for eb in range(2):
                ps = ps_b.tile([128, 512], F32, name="pswo", tag="psb")
                for fc in range(2):
                    nc.tensor.matmul(ps[:], attnT[fc][:, tt * 128:(tt + 1) * 128],
                                     wo_t[:, fc, eb * 512:(eb + 1) * 512],
                                     start=(fc == 0), stop=(fc == 1))
                nc.vector.tensor_copy(ot[:, eb * 512:(eb + 1) * 512], ps[:])
                if split_dma:
                    # tail: overlap the two half-row DMAs on both HW queues
                    eng = nc.sync if eb == 0 else nc.scalar
                    eng.dma_start(
                        io["out"][tt * 128:(tt + 1) * 128, eb * 512:(eb + 1) * 512],
                        ot[:, eb * 512:(eb + 1) * 512])
            if not split_dma:
                nc.sync.dma_start(io["out"][tt * 128:(tt + 1) * 128, :], ot[:])

        # ---------------- schedule ----------------
        # fillers placed by deadline vs DMA arrival (~358GB/s in issue
        # order); (ui, g) -> list of closures
        sched = {
            (0, 0): [lambda: proj_block("k", 0, 1)],
            (0, 2): [lambda: proj_block("k", 0, 2)],
            (0, 4): [lambda: proj_block("k", 0, 3)],
            (0, 6): [lambda: v_tile(0), lambda: v_tile(1)],
            (0, 7): [lambda: v_tile(2), lambda: v_tile(3), lambda: proj_block("q", 0, 1)],
            (1, 0): [lambda: v_tile(4), lambda: v_tile(5)],
            (1, 1): [lambda: v_tile(6), lambda: v_tile(7)],
            (1, 2): [lambda: v_tile(8), lambda: v_tile(9)],
            (1, 3): [lambda: v_tile(10), lambda: v_tile(11)],
            (1, 4): [lambda: v_tile(12), lambda: v_tile(13)],
            (1, 5): [lambda: v_tile(14), lambda: v_tile(15)],
            (1, 7): [lambda: proj_block("q", 0, 2)],
            (2, 1): [lambda: proj_block("k", 1, 0)],
            (2, 3): [lambda: proj_block("k", 1, 1)],
            (2, 4): [lambda: proj_block("q", 0, 3)],
            (2, 5): [lambda: proj_block("k", 1, 2)],
            (2, 7): [lambda: proj_block("k", 1, 3)],
            (3, 3): [lambda: proj_block("q", 1, 0)],
            (3, 5): [lambda: proj_block("q", 1, 1)],
            (4, 3): [lambda: proj_block("q", 1, 2)],
            (4, 5): [lambda: proj_block("q", 1, 3)],
            (5, 2): [lambda: wo_group(0)],
            (5, 4): [lambda: wo_group(1)],
            (5, 6): [lambda: wo_group(2)],
            (6, 2): [lambda: wo_group(3), lambda: wo_group(4)],
            (6, 4): [lambda: wo_group(5)],
            (6, 6): [lambda: wo_group(6)],
            (7, 2): [lambda: wo_group(7), lambda: wo_group(8)],
            (7, 4): [lambda: wo_group(9)],
            (7, 6): [lambda: wo_group(10)],
        }

        # PV FIFO: chunk (u, G) may emit once exp (u, G+1) is emitted
        # (lag-1) and v2[G] is fully projected; chain(u) follows chunk(u,7).
        pv_q = deque()
        emitted = set()
        vcount = [0]

        def pv_ready(item, ui, g):
            kind = item[0]
            if kind == "chain":
                return True
            _, u, G = item
            if vcount[0] < 2 * (G + 1):
                return False
            need = (u, G + 1) if G < 7 else ((u + 1, 0) if u < 7 else None)
            return need is None or need in emitted

        def drain_pv(ui, g, cap=2):
            n = 0
            while pv_q and n < cap:
                item = pv_q[0]
                if not pv_ready(item, ui, g):
                    break
                pv_q.popleft()
                if item[0] == "chain":
                    chain(item[1])
                else:
                    pv_chunk(item[1], item[2])
                    n += 1

        # ---------------- emission ----------------
        warmup(8)
        proj_block("k", 0, 0, pace=2)
        proj_block("q", 0, 0, pace=2)

        orig_vtile = v_tile

        def v_tile_counted(tt):
            orig_vtile(tt)
            vcount[0] += 1

        v_tile = v_tile_counted
        # patch sched closures to use counted v_tile: rebuild lazily instead
        # (closures above captured the name `v_tile` at call time in this
        # scope, so they already see the counted version)

        for ui, (p, blk) in enumerate(units):
            for G in range(8):
                pv_q.append(("chunk", ui, G))
            pv_q.append(("chain", ui))
            for g in range(8):
                etile = epool.tile([128, 2 * 1024], act_dt, name="exp", tag="exp")
                etiles[(ui, g)] = etile
                for j2 in range(2):
                    tk = g * 2 + j2
                    ps = ps_a.tile([128, 1024], F32, name="psa", tag="psa")
                    for hh in range(2):
                        pp = hh * 64
                        nc.tensor.matmul(
                            ps[:, hh * 512:(hh + 1) * 512],
                            kcT[p][pp:pp + 64, tk * 128:(tk + 1) * 128],
                            qcT[p][pp:pp + 64, blk * 512:blk * 512 + 512],
                            start=True, stop=True)
                    nc.scalar.activation(
                        etile[:, j2 * 1024:(j2 + 1) * 1024], ps[:],
                        mybir.ActivationFunctionType.Exp, scale=1.0 / SCALE)
                emitted.add((ui, g))
                drain_pv(ui, g)
                for fn in sched.get((ui, g), ()):
                    fn()
        # tail: flush remaining PV chunks + chain; keep the PE warm with
        # dummy matmuls while the normalize chain runs, then the last Wo
        # block with DMAs split across both HW queues.
        while pv_q:
            item = pv_q.popleft()
            if item[0] == "chain":
                chain(item[1])
            else:
                pv_chunk(item[1], item[2])
        wo_group(11)
        warmup(10)
        for tt in (12, 13, 14, 15):
            wo_group(tt, split_dma=True)


def _build():
    nc = bacc.Bacc("TRN2", target_bir_lowering=False, debug=False)
    io = {}
    for name, shape, dt in (
        ("xq", [128, KC, T], BF),
        ("xk", [128, KC, T], BF),
        ("xv", [128, KC, T], BF),
        ("wq", [128, KC, F], BF),
        ("wk", [128, KC, F], BF),
        ("wv", [128, KC, F], BF),
        ("wo", [128, 2, E], BF),
        ("bq", [128, 2], F32),
        ("bk", [128, 2], F32),
        ("bv", [1, F], F32),
    ):
        io[name] = nc.dram_tensor(name, shape, dt, kind="ExternalInput").ap()
    io["out"] = nc.dram_tensor("out", [T, E], BF, kind="ExternalOutput").ap()
    with tile.TileContext(nc) as tc:
        _emit(tc, io)
    nc.compile()
    return nc


def _fold_clr(W, b, clr):
    """q_c = q - mean_head(q) + clr  ==  x @ (C W).T + (C b + clr)."""
    W64 = W.astype(np.float64).reshape(H, Dh, E)
    W_eff = W64 - W64.mean(axis=1, keepdims=True)
    b64 = b.astype(np.float64).reshape(H, Dh)
    b_eff = b64 - b64.mean(axis=1, keepdims=True) + clr.astype(np.float64).reshape(H, Dh)
    return W_eff.reshape(E, E), b_eff.reshape(E)


_NC_CACHE = None


def _chunk3(a, nchunk):
    """[nchunk*128, M] -> [128, nchunk, M]"""
    n, m = a.shape
    return np.ascontiguousarray(
        a.reshape(nchunk, 128, m).transpose(1, 0, 2))


def kernel(**inputs):
    global _NC_CACHE
    query = np.asarray(inputs["query"], np.float32)
    key = np.asarray(inputs["key"], np.float32)
    value = np.asarray(inputs["value"], np.float32)
    mask = np.asarray(inputs["key_padding_mask"])
    Wq, bq = np.asarray(inputs["Wq"], np.float32), np.asarray(inputs["bq"], np.float32)
    Wk, bk = np.asarray(inputs["Wk"], np.float32), np.asarray(inputs["bk"], np.float32)
    Wv, bv = np.asarray(inputs["Wv"], np.float32), np.asarray(inputs["bv"], np.float32)
    Wo, bo = np.asarray(inputs["Wo"], np.float32), np.asarray(inputs["bo"], np.float32)
    cq = np.asarray(inputs["clr_bias_q"], np.float32)
    ck = np.asarray(inputs["clr_bias_k"], np.float32)
    assert not mask.any(), "kernel assumes empty key_padding_mask"

    Wq_eff, bq_eff = _fold_clr(Wq, bq, cq)
    Wk_eff, bk_eff = _fold_clr(Wk, bk, ck)

    def bf(x):
        return np.ascontiguousarray(x.astype(np.float32)).astype(BF_NP)

    in_maps = []
    for c in range(NCORES):
        b = c // 4
        fs = (c % 4) * F
        m = {
            "xq": bf(_chunk3(query[b].T, KC)),
            "xk": bf(_chunk3(key[b].T, KC)),
            "xv": bf(_chunk3(value[b].T, KC)),
            "wq": bf(_chunk3(Wq_eff[fs:fs + F].T, KC)),
            "wk": bf(_chunk3(Wk_eff[fs:fs + F].T, KC)),
            "wv": bf(_chunk3(Wv[fs:fs + F].T, KC)),
            "wo": bf(_chunk3(Wo[:, fs:fs + F].T, 2)),
            "bq": np.ascontiguousarray(
                bq_eff[fs:fs + F].reshape(2, 128).T.astype(np.float32)),
            "bk": np.ascontiguousarray(
                bk_eff[fs:fs + F].reshape(2, 128).T.astype(np.float32)),
            "bv": np.ascontiguousarray(bv[None, fs:fs + F], dtype=np.float32),
        }
        in_maps.append(m)

    if _NC_CACHE is None:
        _NC_CACHE = _build()
    nc = _NC_CACHE

    import os

    trace = bool(int(os.environ.get("KERNEL_TRACE", "0")))
    if trace:
        _install_ntff_hook()
    res = None
    last_exc = None
    out = None
    for attempt in range(4):
        try:
            res = run_bass_kernel_spmd(
                nc, in_maps, core_ids=list(range(NCORES)), trace=trace
            )
        except Exception as e:  # transient NRT_EXEC_UNIT_UNRECOVERABLE etc.
            last_exc = e
            import time

            time.sleep(2.0)
            continue
        out = np.zeros((B, T, E), np.float32)
        for c in range(NCORES):
            out[c // 4] += res.results[c]["out"].astype(np.float32)
        if np.isfinite(out).all():
            break
        out = None  # rare transient corruption: retry
    if out is None:
        if last_exc is not None and res is None:
            raise last_exc
        raise RuntimeError("kernel produced non-finite output on all attempts")
    kernel.last_results = res
    out += bo[None, None, :].astype(np.float32)
    return out
